# revision 51
# baseline (speedup 1.0000x reference)
"""Trainium2 Bass kernel for MultiHeadAttention (B=8, S=1024, D=1024, H=16, DK=DV=64).

Sharding: data-parallel over batch — each of the 8 NeuronCores computes one
full batch element (QKV projections, masked+scaled softmax attention, output
projection, LayerNorm). No collectives.

Per-core math (batch b), matmul datapath in bf16 (f32 PSUM accumulate):
  Qt = (Wq/8)^T Xq^T          [hd, s]   (head-dim-major / transposed)
  Kt = Wk^T Xk^T              [hd, s]
  V' = Xv Wv (+ ones col)     [s, h*65]
  scores^T = K_h Q_h^T        [k, q] per head
  t = scores^T * mT           (mT = (matrix * !mask).T; masked entries -> 0)
  p = exp(t)                  (masked entries become exp(0) = 1)
  ctx^T = V'_h^T (p^T - maskT)  [65, q]  (row 64 = corrected denominator;
                               the -maskT matmul accumulates into the same
                               PSUM group and exactly removes the masked
                               exp(0)=1 contributions)
  renorm: DVE reciprocal of the denom row -> rank-1 PE broadcast matmul ->
          fused PSUM*recip eviction multiply on DVE (no gpsimd, no tiny
          DMAs; odd heads DMA-shift to partitions 64-127), emitted one
          head-group late so the tensor engine never stalls on it
  out = LN(Ctx Wfc / denom) * gamma + beta
"""
from contextlib import ExitStack

import numpy as np

import concourse.bass as bass
import concourse.bacc as bacc
import concourse.tile as tile
import concourse.mybir as mybir
from concourse.bass_utils import run_bass_kernel_spmd
from concourse.masks import make_identity

F32 = mybir.dt.float32
F32R = mybir.dt.float32r
BF16 = mybir.dt.bfloat16
AF = mybir.ActivationFunctionType
ALU = mybir.AluOpType

B, S, DM, H, DK = 8, 1024, 1024, 16, 64
P = 128
ST = S // P      # seq tiles (8)
DT = DM // P     # d_model tiles (8)
QB = 512         # q-block width in attention phase
NQB = S // QB
HPG = 2          # heads per PSUM group
SC_BUFS = 4      # scores psum depth
CTX_BUFS = 2     # ctx psum depth
T4_BUFS = 3      # t4 depth (DVE->ACT hop)
P4_BUFS = 3      # p4 depth (ACT->PE hop)
LN_EPS = 1e-5


def r32(ap):
    return ap.bitcast(F32R)


def build_bass(apply_gamma_beta: bool, timing_reps: int = 0, phases: str = "ABCD",
               ablate: str = ""):
    nc = bacc.Bacc("TRN2", target_bir_lowering=False, debug=False,
                   enable_asserts=False, num_devices=8)

    timing = timing_reps > 0
    kind = "Internal" if timing else "ExternalInput"

    def dram_in(name, shape, dt):
        if timing:
            return nc.dram_tensor(name, shape, dt).ap()
        return nc.dram_tensor(name, shape, dt, kind="ExternalInput").ap()

    xq = dram_in("xq", [S, DM], BF16)
    xk = dram_in("xk", [S, DM], BF16)
    xv = dram_in("xv", [S, DM], BF16)
    mt = dram_in("mt", [S, S], BF16)         # (matrix*keep)^T [k,q]
    maskt = dram_in("maskt", [S, S], BF16)   # mask^T as float [k,q]
    wq = dram_in("wq", [DM, DM], BF16)       # pre-scaled by 1/sqrt(DK)
    wk = dram_in("wk", [DM, DM], BF16)
    wv = dram_in("wv", [DM, DM], BF16)
    wfc = dram_in("wfc", [DM, DM], BF16)
    if timing:
        out = nc.dram_tensor("out", [S, DM], F32).ap()
        done = nc.dram_tensor("done", [1, 1], F32, kind="ExternalOutput").ap()
    else:
        out = nc.dram_tensor("out", [S, DM], F32, kind="ExternalOutput").ap()
    gamma = beta = None
    if apply_gamma_beta:
        gamma = dram_in("gamma", [DM], F32)
        beta = dram_in("beta", [DM], F32)

    mt_t = mt.rearrange("(t p) q -> p t q", p=P)
    maskt_t = maskt.rearrange("(t p) q -> p t q", p=P)

    with tile.TileContext(nc) as tc, ExitStack() as ctx:
        if timing:
            ctx.enter_context(tc.For_i(0, timing_reps, 1))
        const = ctx.enter_context(tc.tile_pool(name="const", bufs=1))
        eps_t = const.tile([P, 1], F32, tag="eps")
        nc.gpsimd.memset(eps_t[:], LN_EPS)

        # Ctx^T lives through C+D; opened first so A-C pools can release
        ctx_pool = ctx.enter_context(tc.tile_pool(name="ctxp", bufs=1))
        ctx_sb = ctx_pool.tile([P, DT, S], BF16, tag="ctx")       # Ctx^T [hd, q]

        # persistent across phases A-C (released before phase D)
        persist_cm = tc.tile_pool(name="persist", bufs=1)
        persist = persist_cm.__enter__()
        qt_sb = persist.tile([P, DT, S], BF16, tag="qt")          # Qt [hd, s]
        kt_sb = persist.tile([P, DT, S], BF16, tag="kt")          # Kt [hd, s]
        vp_sb = persist.tile([P, ST, H * 65], BF16, tag="vp")     # V' [s, h*65]
        vp_view = vp_sb.rearrange("p t (h d) -> p t h d", d=65)

        # ---------- Phase A: transposes + QKV projections ----------
        if "A" in phases:
          with tc.tile_pool(name="xrow", bufs=6) as xrow_pool, \
             tc.tile_pool(name="xT", bufs=1) as xT_pool, \
             tc.tile_pool(name="wload", bufs=6) as w_pool, \
             tc.tile_pool(name="aconst", bufs=1) as aconst, \
             tc.tile_pool(name="tp_psum", bufs=2, space="PSUM") as tp_psum, \
             tc.tile_pool(name="pj_psum", bufs=2, space="PSUM") as pj_psum:

            ident_f = aconst.tile([P, P], F32, tag="identf")
            make_identity(nc, ident_f)
            ident = aconst.tile([P, P], BF16, tag="ident")
            nc.scalar.copy(ident[:], ident_f[:])
            ones_f = aconst.tile([P, P], F32, tag="onesf")
            nc.gpsimd.memset(ones_f[:], 1.0)

            def transpose_input(x_ap):
                """DRAM x [S, DM] -> SBUF x^T [P, DT, S] (partition=dm, free=s)."""
                xT = xT_pool.tile([P, DT, S], BF16, tag="xT")
                for i in range(ST):               # source s-tile
                    xrow = xrow_pool.tile([P, DM], BF16, tag="xrow")
                    nc.sync.dma_start(xrow[:], x_ap[i * P:(i + 1) * P, :])
                    for j0 in range(0, DT, 4):    # 4 dm-tiles per psum bank
                        ps = tp_psum.tile([P, 4, P], BF16, tag="tp")
                        for jj in range(4):
                            nc.tensor.matmul(ps[:, jj, :],
                                             xrow[:, (j0 + jj) * P:(j0 + jj + 1) * P],
                                             ident[:], is_transpose=True)
                        # strided evict: ps [P,4,P] -> xT[:, j0:j0+4, i*P:(i+1)*P]
                        nc.scalar.copy(xT[:, j0:j0 + 4, i * P:(i + 1) * P], ps[:])
                return xT

            def load_w_half(w_ap, half):
                """Stream one column-half of a weight matrix: [P, DT, DM/2]."""
                w_sb = w_pool.tile([P, DT, DM // 2], BF16, tag="w")
                nc.sync.dma_start(
                    w_sb[:],
                    w_ap.rearrange("(t p) n -> p t n", p=P)[
                        :, :, half * (DM // 2):(half + 1) * (DM // 2)])
                return w_sb

            def proj_T(w_sbs, xT, dst):
                """dst[hd, s] = W^T X^T : lhsT = W tiles [dm, hd], rhs = X^T [dm, s]."""
                for wh in range(2):               # W column halves
                    w_sb = w_sbs[wh]
                    for jm2 in range(DT // 2):    # hd out tiles in this half
                        jm = wh * (DT // 2) + jm2
                        pss = [pj_psum.tile([P, 512], F32, tag="pj",
                                            name=f"pj{jm}{sn}")
                               for sn in range(2)]
                        for kc in range(DT):      # stationary shared by 2 mms
                            for sn in range(2):
                                nc.tensor.matmul(
                                    pss[sn][:],
                                    w_sb[:, kc, jm2 * P:(jm2 + 1) * P],
                                    xT[:, kc, sn * 512:(sn + 1) * 512],
                                    start=(kc == 0), stop=(kc == DT - 1))
                        for sn in range(2):
                            nc.scalar.copy(
                                dst[:, jm, sn * 512:(sn + 1) * 512], pss[sn][:])

            # issue all weight loads up front so no projection waits on DMA
            wq_sbs = [load_w_half(wq, wh) for wh in range(2)]
            wk_sbs = [load_w_half(wk, wh) for wh in range(2)]
            wv_sbs = [load_w_half(wv, wh) for wh in range(2)]

            xkT = transpose_input(xk)
            proj_T(wk_sbs, xkT, kt_sb)

            xqT = transpose_input(xq)
            proj_T(wq_sbs, xqT, qt_sb)

            # V projection: natural [s, hd]; lhsT = Xv^T tiles, rhs = Wv halves
            xvT = transpose_input(xv)
            for jm in range(ST):                  # s out tile
                pss = [pj_psum.tile([P, 512], F32, tag="pj", name=f"pv{jm}{wh}")
                       for wh in range(2)]
                for kc in range(DT):              # stationary shared by 2 mms
                    for wh in range(2):
                        nc.tensor.matmul(
                            pss[wh][:],
                            xvT[:, kc, jm * P:(jm + 1) * P],
                            wv_sbs[wh][:, kc, :],
                            start=(kc == 0), stop=(kc == DT - 1))
                for wh in range(2):
                    nc.scalar.copy(
                        vp_view[:, jm, wh * 8:(wh + 1) * 8, 0:64],
                        pss[wh].rearrange("p (h d) -> p h d", d=64))
            nc.scalar.copy(
                vp_view[:, :, :, 64:65],
                ones_f.rearrange("p (t h o) -> p t h o", h=H, o=1))

        if True:

            # ---------- Phase C: attention ----------
            if "C" in phases:
              with tc.tile_pool(name="mstream", bufs=1) as m_pool, \
                 tc.tile_pool(name="mask", bufs=1) as mask_pool, \
                 tc.tile_pool(name="att", bufs=T4_BUFS) as att_pool, \
                 tc.tile_pool(name="attp", bufs=P4_BUFS) as attp_pool, \
                 tc.tile_pool(name="cconst", bufs=1) as cconst, \
                 tc.tile_pool(name="rbp", bufs=2) as stg_pool, \
                 tc.tile_pool(name="dn", bufs=1) as dn_pool, \
                 tc.tile_pool(name="sc_psum", bufs=SC_BUFS, space="PSUM") as sc_psum, \
                 tc.tile_pool(name="ctx_psum", bufs=CTX_BUFS, space="PSUM") as ctx_psum:
                maskt_sb = mask_pool.tile([P, ST, S], BF16, tag="maskt")
                nc.sync.dma_start(maskt_sb[:], maskt_t)
                # all-ones row at partition 64: stationary for the denominator
                # broadcast matmuls (rank-1 outer product with the recip row)
                ones64 = cconst.tile([P, P], F32, tag="ones64")
                nc.vector.memset(ones64[64:65, :], 1.0)
                def renorm_evict(ctx4, qs, hg):
                    # renorm + evict, gpsimd-free: one DVE reciprocal over
                    # both heads' denom rows (partition 64), one rank-1 PE
                    # matmul broadcasting them across 64 partitions, one
                    # PSUM->SBUF copy, then fused PSUM*recip multiplies on
                    # DVE.  Odd heads still DMA-shift to partitions 64-127.
                    rb_sb = dn_pool.tile([P, HPG, QB], F32, tag="rbsb")
                    for hl in range(HPG):
                        den_r = dn_pool.tile([P, QB], F32R, tag="denr")
                        with nc.allow_low_precision(
                                reason="f32r recip feeds broadcast mm"):
                            nc.vector.reciprocal(
                                den_r[64:65, :],
                                ctx4[64:65, hl * QB:(hl + 1) * QB])
                        rb_ps = sc_psum.tile([P, QB], F32, tag="sc")
                        nc.tensor.matmul(
                            rb_ps[0:64, :], r32(ones64[64:65, 0:64]),
                            den_r[64:65, :])
                        nc.vector.tensor_copy(rb_sb[0:64, hl, :],
                                              rb_ps[0:64, :])
                    for hl in range(HPG):
                        h = hg * HPG + hl
                        jt, po = h // 2, (h % 2) * 64
                        cs = ctx4[0:64, hl * QB:(hl + 1) * QB]
                        rbs = rb_sb[0:64, hl, :]
                        if po == 0:
                            nc.vector.tensor_mul(ctx_sb[0:64, jt, qs],
                                                 cs, rbs)
                        else:
                            stg = stg_pool.tile([64, QB], BF16, tag="stg")
                            nc.vector.tensor_mul(stg[:], cs, rbs)
                            nc.sync.dma_start(ctx_sb[64:128, jt, qs], stg[:])

                # renorm/evict of head-group g is emitted one group late so
                # its PE broadcast matmul never stalls the tensor engine:
                # group g+1's score/AV matmuls cover the recip latency.
                pending = None
                for qb in range(NQB):
                    qs = slice(qb * QB, (qb + 1) * QB)
                    mt_qb = m_pool.tile([P, ST, QB], BF16, tag="mtq")
                    nc.sync.dma_start(mt_qb[:], mt_t[:, :, qs])
                    for hg in range(H // HPG):
                        ctx4 = ctx_psum.tile([65, HPG * QB], F32, tag="ctx4")
                        for kc in range(ST):
                            t4 = att_pool.tile([P, HPG, QB], F32, tag="t4")
                            p4 = attp_pool.tile([P, HPG, QB], BF16, tag="p4")
                            for hl in range(HPG):
                                h = hg * HPG + hl
                                jt, po = h // 2, (h % 2) * 64
                                ps_s = sc_psum.tile([P, QB], F32, tag="sc")
                                nc.tensor.matmul(
                                    ps_s[:],
                                    kt_sb[po:po + 64, jt, kc * P:(kc + 1) * P],
                                    qt_sb[po:po + 64, jt, qs])
                                nc.vector.tensor_mul(t4[:, hl, :], ps_s[:],
                                                     mt_qb[:, kc, :])
                            nc.scalar.activation(p4[:], t4[:], AF.Exp)
                            for hl in range(HPG):
                                h = hg * HPG + hl
                                # -mask correction accumulates into the same
                                # PSUM group (removes masked exp(0)=1 terms,
                                # corrects the row-64 denominator) and shares
                                # its stationary operand with the AV matmul
                                nc.tensor.matmul(
                                    ctx4[:, hl * QB:(hl + 1) * QB],
                                    vp_view[:, kc, h, 0:65],
                                    maskt_sb[:, kc, qs],
                                    start=(kc == 0), stop=False)
                                nc.tensor.matmul(
                                    ctx4[:, hl * QB:(hl + 1) * QB],
                                    vp_view[:, kc, h, 0:65],
                                    p4[:, hl, :],
                                    start=False, stop=(kc == ST - 1))
                        if pending is not None:
                            renorm_evict(*pending)
                        pending = (ctx4, qs, hg)
                if pending is not None:
                    renorm_evict(*pending)

            persist_cm.__exit__(None, None, None)

            # ---------- Phase D: output projection + LayerNorm ----------
            if "D" in phases:
              with tc.tile_pool(name="wfc", bufs=1) as wfc_pool, \
                 tc.tile_pool(name="ln", bufs=2) as ln_pool, \
                 tc.tile_pool(name="lnstat", bufs=4) as stat_pool, \
                 tc.tile_pool(name="gb", bufs=1) as gb_pool, \
                 tc.tile_pool(name="fc_psum", bufs=2, space="PSUM") as fc_psum:
                wfc_tiles = []
                for kc in range(DT):
                    wt = wfc_pool.tile([P, DM], BF16, tag=f"wfc{kc}")
                    nc.sync.dma_start(wt[:], wfc[kc * P:(kc + 1) * P, :])
                    wfc_tiles.append(wt)
                if apply_gamma_beta:
                    gm = gb_pool.tile([P, DM], F32, tag="gm")
                    bt = gb_pool.tile([P, DM], F32, tag="bt")
                    nc.sync.dma_start(gm[:], bass.AP(
                        tensor=gamma.tensor, offset=gamma.offset,
                        ap=[[0, P]] + list(gamma.ap)))
                    nc.sync.dma_start(bt[:], bass.AP(
                        tensor=beta.tensor, offset=beta.offset,
                        ap=[[0, P]] + list(beta.ap)))
                inv_d = 1.0 / DM
                for st in range(ST):
                    ps_o = fc_psum.tile([P, DM], F32, tag="fc")
                    for kc in range(DT):          # stationary shared by 2 mms
                        for half in range(2):
                            nc.tensor.matmul(
                                ps_o[:, half * 512:(half + 1) * 512],
                                ctx_sb[:, kc, st * P:(st + 1) * P],
                                wfc_tiles[kc][:, half * 512:(half + 1) * 512],
                                start=(kc == 0), stop=(kc == DT - 1))
                    x_sb = ln_pool.tile([P, DM], F32, tag="x")
                    sq_sb = ln_pool.tile([P, DM], F32, tag="sq")
                    y_sb = ln_pool.tile([P, DM], F32, tag="y")
                    s1 = stat_pool.tile([P, 1], F32, tag="s1")
                    s2 = stat_pool.tile([P, 1], F32, tag="s2")
                    mu = stat_pool.tile([P, 1], F32, tag="mu")
                    var = stat_pool.tile([P, 1], F32, tag="var")
                    std = stat_pool.tile([P, 1], F32, tag="std")
                    rstd = stat_pool.tile([P, 1], F32, tag="rstd")
                    nb = stat_pool.tile([P, 1], F32, tag="nb")
                    nc.scalar.activation(x_sb[:], ps_o[:], AF.Copy, accum_out=s1[:])
                    nc.scalar.activation(sq_sb[:], x_sb[:], AF.Square, accum_out=s2[:])
                    nc.vector.tensor_scalar_mul(mu[:], s1[:], inv_d)
                    # var = s2/D - mu^2
                    nc.vector.scalar_tensor_tensor(var[:], mu[:], 1.0, mu[:],
                                                   ALU.mult, ALU.mult)
                    nc.vector.scalar_tensor_tensor(var[:], s2[:], inv_d, var[:],
                                                   ALU.mult, ALU.subtract)
                    nc.scalar.activation(std[:], var[:], AF.Sqrt, bias=eps_t[:])
                    nc.vector.reciprocal(rstd[:], std[:])
                    nc.vector.scalar_tensor_tensor(nb[:], mu[:], -1.0, rstd[:],
                                                   ALU.mult, ALU.mult)
                    nc.scalar.activation(y_sb[:], x_sb[:], AF.Identity,
                                         bias=nb[:], scale=rstd[:])
                    if apply_gamma_beta:
                        nc.vector.tensor_mul(y_sb[:], y_sb[:], gm[:])
                        nc.vector.tensor_add(y_sb[:], y_sb[:], bt[:])
                    nc.sync.dma_start(out[st * P:(st + 1) * P, :], y_sb[:])

        if timing:
            with tc.tile_pool(name="donep", bufs=1) as dp:
                dt_ = dp.tile([1, 1], F32, tag="done")
                nc.vector.memset(dt_[:], 1.0)
                nc.sync.dma_start(done[:], dt_[:])

    nc.compile()
    return nc


_CACHE = {}


def _get_nc(apply_gamma_beta: bool):
    if apply_gamma_beta not in _CACHE:
        _CACHE[apply_gamma_beta] = build_bass(apply_gamma_beta)
    return _CACHE[apply_gamma_beta]


def _prep(inputs):
    """Build (nc, in_maps) for the SPMD run from the full unsharded inputs."""
    return _prep_impl(**inputs)


def _prep_impl(input_Q, input_K, input_V, attn_mask, matrix, Wq, Wk, Wv, Wfc,
               gamma, beta):
    input_Q = np.ascontiguousarray(np.asarray(input_Q, np.float32))
    input_K = np.ascontiguousarray(np.asarray(input_K, np.float32))
    input_V = np.ascontiguousarray(np.asarray(input_V, np.float32))
    attn_mask = np.asarray(attn_mask)
    matrix = np.asarray(matrix, np.float32)
    Wq = np.ascontiguousarray(np.asarray(Wq, np.float32))
    Wk = np.ascontiguousarray(np.asarray(Wk, np.float32))
    Wv = np.ascontiguousarray(np.asarray(Wv, np.float32))
    Wfc = np.ascontiguousarray(np.asarray(Wfc, np.float32))
    gamma = np.asarray(gamma, np.float32)
    beta = np.asarray(beta, np.float32)

    trivial_gb = bool(np.all(gamma == 1.0) and np.all(beta == 0.0))
    nc = _get_nc(not trivial_gb)

    from ml_dtypes import bfloat16
    wq_s = np.ascontiguousarray(Wq / np.sqrt(DK)).astype(bfloat16)
    wk_b = Wk.astype(bfloat16)
    wv_b = Wv.astype(bfloat16)
    wfc_b = Wfc.astype(bfloat16)
    keep = (~attn_mask).astype(np.float32)           # [B, S, S]
    m_eff = matrix[:, 0, :, :] * keep                # [B, S, S]

    in_maps = []
    for b in range(B):
        im = {
            "xq": input_Q[b].astype(bfloat16),
            "xk": input_K[b].astype(bfloat16),
            "xv": input_V[b].astype(bfloat16),
            "mt": np.ascontiguousarray(m_eff[b].T.astype(bfloat16)),
            "maskt": np.ascontiguousarray(-attn_mask[b].T.astype(bfloat16)),
            "wq": wq_s, "wk": wk_b, "wv": wv_b, "wfc": wfc_b,
        }
        if not trivial_gb:
            im["gamma"] = gamma
            im["beta"] = beta
        in_maps.append(im)
    return nc, in_maps


def kernel(**inputs):
    nc, in_maps = _prep(inputs)
    res = run_bass_kernel_spmd(nc, in_maps, core_ids=list(range(B)))
    return np.stack([res.results[b]["out"] for b in range(B)], axis=0)

